# revision 1
# baseline (speedup 1.0000x reference)
"""Distributed Trainium2 Bass kernel for nn_Attention (LN + fused QKV + RoPE +
MHA-with-in-proj + out-proj), SPMD over 8 NeuronCores.

Sharding: both batches are sequence-sharded across all 8 cores. Core c owns
rows [256c, 256c+256) of batch 0 AND of batch 1 (512 tokens/core). Projections
run on the mixed 512-token block (N=512 matmuls); attention runs per batch
(N=256). K/V heads are exchanged with two single-group 8-core AllGathers
(4-core subgroup collectives hang on this runtime). Output needs no
collective: each core produces final out rows for its tokens.

Layout notes:
 - Everything is feature-major ("T" suffix): tensor[feature, token].
 - RoPE: q/k feature dims are pre-permuted on the host (all even pair members
   first, then all odd) so the rotation becomes elementwise between the two
   halves; the in-projection weights get the matching row permutation.
 - LayerNorm affine (g, b) is folded into the qkv weights on the host; the
   1/sqrt(hd) score scale is folded into wq.
 - Matmuls run in float32r (full-rate fp32 mode, free dim >= 256). The
   attention-value matmul runs in bf16 (attn weights produced in bf16 by the
   ACT exp pass; mask applied multiplicatively as exp(mask)).
"""

import numpy as np

import concourse.bass as bass
import concourse.tile as tile
from concourse import bacc, mybir
from concourse.bass_utils import run_bass_kernel_spmd

B, S, D = 2, 2048, 1024
H, HD = 16, 64
NCORES = 8
TPB = 256  # tokens per core per batch
T = 2 * TPB  # tokens per core
EPS = 1e-5
THETA = 10000.0
P = 128
F32 = mybir.dt.float32
F32R = mybir.dt.float32r
BF16 = mybir.dt.bfloat16
Copy = mybir.ActivationFunctionType.Copy
Ident = mybir.ActivationFunctionType.Identity
Exp = mybir.ActivationFunctionType.Exp
Rsqrt = mybir.ActivationFunctionType.Rsqrt
MUL = mybir.AluOpType.mult
ADD = mybir.AluOpType.add
SUB = mybir.AluOpType.subtract

TRACE = False  # test.py flips this for profiling runs

_cached = {}


def _build_module():
    nc = bacc.Bacc(None, target_bir_lowering=False)

    xT = nc.declare_dram_parameter("xT", [D, T], F32R, isOutput=False)
    maskT = nc.declare_dram_parameter("maskT", [S, T], F32, isOutput=False)
    cosT = nc.declare_dram_parameter("cosT", [D // 2, T], F32, isOutput=False)
    sinT = nc.declare_dram_parameter("sinT", [D // 2, T], F32, isOutput=False)
    w1qkT = nc.declare_dram_parameter("w1qkT", [D, 2 * D], F32R, isOutput=False)
    w1vT = nc.declare_dram_parameter("w1vT", [D, D], F32R, isOutput=False)
    b1qk = nc.declare_dram_parameter("b1qk", [2 * D], F32, isOutput=False)
    b1v = nc.declare_dram_parameter("b1v", [D], F32, isOutput=False)
    w2T = nc.declare_dram_parameter("w2T", [D, 2 * D], F32R, isOutput=False)
    b2q = nc.declare_dram_parameter("b2q", [D], F32, isOutput=False)
    b2k = nc.declare_dram_parameter("b2k", [D], F32, isOutput=False)
    wvT = nc.declare_dram_parameter("wvT", [D, D], F32R, isOutput=False)
    bvr = nc.declare_dram_parameter("bvr", [1, D], F32R, isOutput=False)
    owT = nc.declare_dram_parameter("owT", [D, D], F32R, isOutput=False)
    outb = nc.declare_dram_parameter("outb", [D], F32, isOutput=False)
    outT = nc.declare_dram_parameter("outT", [D, T], F32, isOutput=True)

    RG = [list(range(NCORES))]

    with tile.TileContext(nc) as tc:
        with (
            tc.tile_pool(name="persist", bufs=1) as persist,
            tc.tile_pool(name="dram", bufs=1, space="DRAM") as dram,
        ):
            qhT = persist.tile([HD, H, T], F32R)  # [hd, h, t]
            avT = persist.tile([P, 8, T], F32R)  # attention output, feature-major
            expm = persist.tile([P, 16, T], BF16)  # exp(mask), key-major
            b1qk_sb = persist.tile([P, 16], F32)
            b1v_sb = persist.tile([P, 8], F32)
            b2q_sb = persist.tile([HD, H], F32)
            b2k_sb = persist.tile([P, 8], F32)
            outb_sb = persist.tile([P, 8], F32)
            bvr_sb = persist.tile([1, D], F32R)
            ones_col = persist.tile([P, 1], F32R)
            ones_row = persist.tile([1, P], F32R)
            eps_sb = persist.tile([1, 1], F32)

            ones_f32 = persist.tile([P, 1], F32)
            nc.vector.memset(ones_f32[:], 1.0)
            nc.vector.tensor_scalar_mul(ones_col[:], ones_f32[:], 1.0)
            ones_row_f = persist.tile([1, P], F32)
            nc.vector.memset(ones_row_f[:], 1.0)
            nc.vector.tensor_scalar_mul(ones_row[:], ones_row_f[:], 1.0)
            nc.vector.memset(eps_sb[:], EPS)
            nc.sync.dma_start(b1qk_sb[:], b1qk.rearrange("(o p) -> p o", p=P))
            nc.sync.dma_start(b1v_sb[:], b1v.rearrange("(o p) -> p o", p=P))
            nc.sync.dma_start(b2q_sb[:], b2q.rearrange("(h p) -> p h", p=HD))
            nc.sync.dma_start(b2k_sb[:], b2k.rearrange("(o p) -> p o", p=P))
            nc.sync.dma_start(outb_sb[:], outb.rearrange("(o p) -> p o", p=P))
            nc.sync.dma_start(bvr_sb[:], bvr[:])

            ag1_in = dram.tile([D, T], F32R)
            ag1_out = dram.tile([NCORES * D, T], F32R, addr_space="Shared")
            ag2_in = dram.tile([T, D], BF16)
            ag2_out = dram.tile([NCORES * T, D], BF16, addr_space="Shared")

            # ---- exp(mask) (ACT, overlaps the projection phase) ----
            with tc.tile_pool(name="mload", bufs=2) as mload:
                mview = maskT.rearrange("(jc p) t -> p jc t", p=P)
                for g in range(4):
                    mt = mload.tile([P, 4, T], F32)
                    nc.sync.dma_start(mt[:], mview[:, 4 * g : 4 * g + 4, :])
                    nc.scalar.activation(
                        out=expm[:, 4 * g : 4 * g + 4, :], in_=mt[:], func=Exp
                    )

            with tc.tile_pool(name="wpool", bufs=3) as wpool:
                with tc.tile_pool(name="xpool", bufs=1) as xpool:
                    xfull = xpool.tile([P, 8, T], F32R)
                    xnT = xpool.tile([P, 8, T], F32R)
                    nc.sync.dma_start(
                        xfull[:], xT.rearrange("(ko p) t -> p ko t", p=P)
                    )

                    # ---- LayerNorm ----
                    with (
                        tc.tile_pool(name="lnt", bufs=3) as lnt,
                        tc.tile_pool(name="lnrows", bufs=1) as lnrows,
                        tc.tile_pool(name="psLN", bufs=2, space="PSUM") as psLN,
                    ):
                        pt_s = psLN.tile([P, T], F32)
                        pt_q = psLN.tile([P, T], F32)
                        for ko in range(8):
                            sq = lnt.tile([P, T], F32R)
                            nc.vector.tensor_tensor(
                                sq[:], xfull[:, ko, :], xfull[:, ko, :], MUL
                            )
                            nc.tensor.matmul(
                                pt_s[0:1, :],
                                ones_col[:],
                                xfull[:, ko, :],
                                start=(ko == 0),
                                stop=(ko == 7),
                            )
                            nc.tensor.matmul(
                                pt_q[0:1, :],
                                ones_col[:],
                                sq[:],
                                start=(ko == 0),
                                stop=(ko == 7),
                            )
                        mu = lnrows.tile([1, T], F32)
                        msq = lnrows.tile([1, T], F32)
                        nc.scalar.activation(
                            out=mu[:], in_=pt_s[0:1, :], func=Copy, scale=1.0 / D
                        )
                        nc.scalar.activation(
                            out=msq[:], in_=pt_q[0:1, :], func=Copy, scale=1.0 / D
                        )
                        var = lnrows.tile([1, T], F32)
                        nc.vector.tensor_tensor(var[:], mu[:], mu[:], MUL)
                        nc.vector.tensor_tensor(var[:], msq[:], var[:], SUB)
                        sd = lnrows.tile([1, T], F32)
                        nc.scalar.activation(
                            out=sd[:], in_=var[:],
                            func=mybir.ActivationFunctionType.Sqrt,
                            bias=eps_sb[:],
                        )
                        rstd = lnrows.tile([1, T], F32)
                        nc.vector.reciprocal(rstd[:], sd[:])
                        murstd = lnrows.tile([1, T], F32)
                        nc.vector.tensor_tensor(murstd[:], mu[:], rstd[:], MUL)
                        rstd_b = lnrows.tile([P, T], F32)
                        murstd_b = lnrows.tile([P, T], F32)
                        nc.gpsimd.partition_broadcast(rstd_b[:], rstd[:])
                        nc.gpsimd.partition_broadcast(murstd_b[:], murstd[:])
                        for ko in range(8):
                            t1 = lnt.tile([P, T], F32, tag="t1")
                            nc.vector.tensor_tensor(
                                t1[:], xfull[:, ko, :], rstd_b[:], MUL
                            )
                            nc.vector.tensor_tensor(
                                xnT[:, ko, :], t1[:], murstd_b[:], SUB
                            )

                    w1view = w1qkT.rearrange("(ko p) j -> p ko j", p=P)
                    w1vview = w1vT.rearrange("(ko p) j -> p ko j", p=P)
                    w2view = w2T.rearrange("(ko p) j -> p ko j", p=P)

                    with tc.tile_pool(name="psA", bufs=2, space="PSUM") as psA:
                        # ---- k chain: project k, rope, in-proj kh, AllGather ----
                        with (
                            tc.tile_pool(name="qk", bufs=1) as qkp,
                            tc.tile_pool(name="rope", bufs=1) as ropep,
                            tc.tile_pool(name="rtmp", bufs=2) as rtmp,
                            tc.tile_pool(name="khp", bufs=1) as khp,
                            tc.tile_pool(name="cs", bufs=1) as csp,
                        ):
                            cos_sb = csp.tile([P, 4, T], F32)
                            sin_sb = csp.tile([P, 4, T], F32)
                            nc.sync.dma_start(
                                cos_sb[:], cosT.rearrange("(c p) t -> p c t", p=P)
                            )
                            nc.sync.dma_start(
                                sin_sb[:], sinT.rearrange("(c p) t -> p c t", p=P)
                            )

                            def project(dst, dst_ko, wview, jcol, bias, rhs):
                                wt = wpool.tile([P, 8, P], F32R, tag="w")
                                nc.sync.dma_start(
                                    wt[:], wview[:, :, jcol : jcol + P]
                                )
                                pt = psA.tile([P, T], F32, tag="proj")
                                for ko in range(8):
                                    nc.tensor.matmul(
                                        pt[:],
                                        wt[:, ko, :],
                                        rhs[:, ko, :],
                                        start=(ko == 0),
                                        stop=(ko == 7),
                                    )
                                nc.scalar.activation(
                                    out=dst[:, dst_ko, :],
                                    in_=pt[:],
                                    func=Ident,
                                    bias=bias,
                                )

                            def rope(dst, src):
                                for c in range(4):
                                    x1 = src[:, c, :]
                                    x2 = src[:, 4 + c, :]
                                    ta = rtmp.tile([P, T], F32, tag="ra")
                                    tb = rtmp.tile([P, T], F32, tag="rb")
                                    nc.vector.tensor_tensor(
                                        ta[:], x1, cos_sb[:, c, :], MUL
                                    )
                                    nc.vector.tensor_tensor(
                                        tb[:], x2, sin_sb[:, c, :], MUL
                                    )
                                    nc.vector.tensor_tensor(
                                        dst[:, c, :], ta[:], tb[:], SUB
                                    )
                                    tc2 = rtmp.tile([P, T], F32, tag="ra")
                                    td = rtmp.tile([P, T], F32, tag="rb")
                                    nc.vector.tensor_tensor(
                                        tc2[:], x2, cos_sb[:, c, :], MUL
                                    )
                                    nc.vector.tensor_tensor(
                                        td[:], x1, sin_sb[:, c, :], MUL
                                    )
                                    nc.vector.tensor_tensor(
                                        dst[:, 4 + c, :], tc2[:], td[:], ADD
                                    )

                            kT = qkp.tile([P, 8, T], F32, tag="qk")
                            for jm in range(8):
                                project(
                                    kT, jm, w1view, D + P * jm,
                                    b1qk_sb[:, 8 + jm : 9 + jm], xnT,
                                )
                            rk = ropep.tile([P, 8, T], F32R, tag="rope")
                            rope(rk, kT)
                            khT_tmp = khp.tile([P, 8, T], F32R)
                            for jm in range(8):
                                project(
                                    khT_tmp, jm, w2view, D + P * jm,
                                    b2k_sb[:, jm : jm + 1], rk,
                                )
                            nc.sync.dma_start(
                                ag1_in.rearrange("(ko p) t -> p ko t", p=P),
                                khT_tmp[:],
                            )
                            nc.gpsimd.collective_compute(
                                "AllGather",
                                mybir.AluOpType.bypass,
                                ins=[ag1_in.opt()],
                                outs=[ag1_out.opt()],
                                replica_groups=RG,
                            )

                            # ---- q chain ----
                            qT = qkp.tile([P, 8, T], F32, tag="qk")
                            for jm in range(8):
                                project(
                                    qT, jm, w1view, P * jm,
                                    b1qk_sb[:, jm : jm + 1], xnT,
                                )
                            rq = ropep.tile([P, 8, T], F32R, tag="rope")
                            rope(rq, qT)
                            # qh: per-head M=64 matmuls so each head's slice
                            # starts at partition 0
                            for h in range(H):
                                wt = wpool.tile([P, 8, HD], F32R, tag="wq")
                                nc.sync.dma_start(
                                    wt[:], w2view[:, :, HD * h : HD * h + HD]
                                )
                                pt = psA.tile([P, T], F32, tag="proj")
                                for ko in range(8):
                                    nc.tensor.matmul(
                                        pt[0:HD, :],
                                        wt[:, ko, :],
                                        rq[:, ko, :],
                                        start=(ko == 0),
                                        stop=(ko == 7),
                                    )
                                nc.scalar.activation(
                                    out=qhT[:, h, :],
                                    in_=pt[0:HD, :],
                                    func=Ident,
                                    bias=b2q_sb[:, h : h + 1],
                                )

                        # ---- v chain ----
                        with tc.tile_pool(name="vp", bufs=1) as vp:
                            vT = vp.tile([P, 8, T], F32R)
                            for jm in range(8):
                                wt = wpool.tile([P, 8, P], F32R, tag="w")
                                nc.sync.dma_start(
                                    wt[:], w1vview[:, :, P * jm : P * jm + P]
                                )
                                pt = psA.tile([P, T], F32, tag="proj")
                                for ko in range(8):
                                    nc.tensor.matmul(
                                        pt[:],
                                        wt[:, ko, :],
                                        xnT[:, ko, :],
                                        start=(ko == 0),
                                        stop=(ko == 7),
                                    )
                                nc.scalar.activation(
                                    out=vT[:, jm, :],
                                    in_=pt[:],
                                    func=Ident,
                                    bias=b1v_sb[:, jm : jm + 1],
                                )

                            # vh (token-major) = vT.T @ wvT + bv, in bf16
                            with (
                                tc.tile_pool(name="wvp", bufs=2) as wvp,
                                tc.tile_pool(name="vhp", bufs=1) as vhp,
                            ):
                                vh_bf = vhp.tile([P, 4, D], BF16)
                                wvview = wvT.rearrange("(ko p) n -> p ko n", p=P)
                                for nh in range(2):
                                    wv_rhs = wvp.tile([P, 8, 512], F32R)
                                    nc.sync.dma_start(
                                        wv_rhs[:],
                                        wvview[:, :, 512 * nh : 512 * nh + 512],
                                    )
                                    for tm in range(4):
                                        pt = psA.tile([P, T], F32, tag="proj")
                                        for ko in range(8):
                                            nc.tensor.matmul(
                                                pt[:, 0:512],
                                                vT[:, ko, P * tm : P * tm + P],
                                                wv_rhs[:, ko, :],
                                                start=(ko == 0),
                                                stop=False,
                                            )
                                        nc.tensor.matmul(
                                            pt[:, 0:512],
                                            ones_row[:],
                                            bvr_sb[0:1, 512 * nh : 512 * nh + 512],
                                            start=False,
                                            stop=True,
                                        )
                                        nc.vector.tensor_copy(
                                            vh_bf[:, tm, 512 * nh : 512 * nh + 512],
                                            pt[:, 0:512],
                                        )
                                nc.sync.dma_start(
                                    ag2_in.rearrange("(tm p) n -> p tm n", p=P),
                                    vh_bf[:],
                                )
                                nc.gpsimd.collective_compute(
                                    "AllGather",
                                    mybir.AluOpType.bypass,
                                    ins=[ag2_in.opt()],
                                    outs=[ag2_out.opt()],
                                    replica_groups=RG,
                                )

                        # ---- attention ----
                        # ag1_out rows: 1024*r + (64*h + hd); cols 256*b + i
                        # ag2_out rows: 512*r + 256*b + tok ; cols 64*h + hd
                        kview = ag1_out.rearrange(
                            "(r hh hd) t -> hh hd r t", hh=H, hd=HD
                        )
                        vview = ag2_out.rearrange(
                            "(r b2 half p) f -> b2 half p r f", b2=2, half=2, p=P
                        )
                        with (
                            tc.tile_pool(name="kload", bufs=2) as kload,
                            tc.tile_pool(name="vload", bufs=2) as vload,
                            tc.tile_pool(name="apool", bufs=3) as apool,
                            tc.tile_pool(name="nrm", bufs=2) as nrm,
                            tc.tile_pool(name="psS", bufs=2, space="PSUM") as psS,
                            tc.tile_pool(name="psV", bufs=2, space="PSUM") as psV,
                        ):
                            for b in range(2):
                                for h in range(H):
                                    kh_sb = kload.tile([HD, 8, TPB], F32R)
                                    nc.sync.dma_start(
                                        kh_sb[:],
                                        kview[h][:, :, TPB * b : TPB * b + TPB],
                                    )
                                    vh_sb = vload.tile([P, 8, 2, HD + 1], BF16)
                                    for half in range(2):
                                        nc.sync.dma_start(
                                            vh_sb[:, :, half, 0:HD],
                                            vview[b].rearrange(
                                                "half p r f -> half p r f"
                                            )[half][:, :, HD * h : HD * h + HD],
                                        )
                                    nc.vector.memset(vh_sb[:, :, :, HD : HD + 1], 1.0)

                                    av_pt = psV.tile([P, TPB], F32)
                                    for g in range(4):
                                        s_pt = psS.tile([P, 4, TPB], F32)
                                        for u in range(4):
                                            jc = 4 * g + u
                                            r, half = jc // 2, jc % 2
                                            nc.tensor.matmul(
                                                s_pt[:, u, :],
                                                kh_sb[
                                                    :, r, P * half : P * half + P
                                                ],
                                                qhT[:, h, TPB * b : TPB * b + TPB],
                                                start=True,
                                                stop=True,
                                            )
                                        attnE = apool.tile(
                                            [P, 4, TPB], BF16, tag="ae"
                                        )
                                        nc.scalar.activation(
                                            out=attnE[:], in_=s_pt[:], func=Exp
                                        )
                                        attnT = apool.tile(
                                            [P, 4, TPB], BF16, tag="at"
                                        )
                                        nc.vector.tensor_tensor(
                                            attnT[:],
                                            attnE[:],
                                            expm[
                                                :, 4 * g : 4 * g + 4,
                                                TPB * b : TPB * b + TPB
                                            ],
                                            MUL,
                                        )
                                        for u in range(4):
                                            jc = 4 * g + u
                                            nc.tensor.matmul(
                                                av_pt[0 : HD + 1, :],
                                                vh_sb[:, jc // 2, jc % 2, :],
                                                attnT[:, u, :],
                                                start=(g == 0 and u == 0),
                                                stop=(g == 3 and u == 3),
                                            )
                                    # normalize by the ones-row denominator
                                    avs = nrm.tile([P, TPB], F32, tag="avs")
                                    nc.scalar.activation(
                                        out=avs[0 : HD + 1, :],
                                        in_=av_pt[0 : HD + 1, :],
                                        func=Copy,
                                    )
                                    drow = nrm.tile([1, TPB], F32, tag="dr")
                                    nc.sync.dma_start(
                                        drow[:], avs[HD : HD + 1, :]
                                    )
                                    rrow = nrm.tile([1, TPB], F32, tag="rr")
                                    nc.vector.reciprocal(rrow[:], drow[:])
                                    rb = nrm.tile([HD, TPB], F32, tag="rbt")
                                    nc.gpsimd.partition_broadcast(rb[:], rrow[:])
                                    if h % 2 == 0:
                                        nc.vector.tensor_tensor(
                                            avT[
                                                0:HD, h // 2,
                                                TPB * b : TPB * b + TPB
                                            ],
                                            avs[0:HD, :],
                                            rb[:],
                                            MUL,
                                        )
                                    else:
                                        avn = nrm.tile([HD, TPB], F32R, tag="avn")
                                        nc.vector.tensor_tensor(
                                            avn[:], avs[0:HD, :], rb[:], MUL
                                        )
                                        nc.sync.dma_start(
                                            avT[
                                                HD:P, h // 2,
                                                TPB * b : TPB * b + TPB
                                            ],
                                            avn[:],
                                        )

                        # ---- output projection ----
                        with tc.tile_pool(name="op", bufs=1) as op:
                            outT_sb = op.tile([P, 8, T], F32)
                            owview = owT.rearrange("(ko p) j -> p ko j", p=P)
                            for om in range(8):
                                pt = psA.tile([P, T], F32, tag="proj")
                                for ko in range(8):
                                    wt = wpool.tile([P, 8, P], F32R, tag="w")
                                    if ko == 0:
                                        nc.sync.dma_start(
                                            wt[:], owview[:, :, P * om : P * om + P]
                                        )
                                        wth = wt
                                    nc.tensor.matmul(
                                        pt[:],
                                        wth[:, ko, :],
                                        avT[:, ko, :],
                                        start=(ko == 0),
                                        stop=(ko == 7),
                                    )
                                nc.scalar.activation(
                                    out=outT_sb[:, om, :],
                                    in_=pt[:],
                                    func=Ident,
                                    bias=outb_sb[:, om : om + 1],
                                )
                            nc.sync.dma_start(
                                outT.rearrange("(ko p) t -> p ko t", p=P),
                                outT_sb[:],
                            )

    nc.finalize()
    return nc


def _host_prep(x, mask, ln_g, ln_b, w_qkv, b_qkv, in_w, in_b, out_w, out_b):
    f32 = np.float32
    perm = np.concatenate([np.arange(0, D, 2), np.arange(1, D, 2)])
    W1 = (w_qkv * ln_g[None, :]).astype(f32)
    b1 = (b_qkv + w_qkv @ ln_b).astype(f32)
    W1q, W1k, W1v = W1[0:D], W1[D : 2 * D], W1[2 * D :]
    b1q, b1k, b1v = b1[0:D], b1[D : 2 * D], b1[2 * D :]
    w1qkT = np.ascontiguousarray(
        np.concatenate([W1q[perm], W1k[perm]], axis=0).T
    ).astype(f32)
    b1qk = np.concatenate([b1q[perm], b1k[perm]]).astype(f32)
    w1vT = np.ascontiguousarray(W1v.T).astype(f32)

    wq, wk, wv = in_w[0:D], in_w[D : 2 * D], in_w[2 * D :]
    bq, bk, bv = in_b[0:D], in_b[D : 2 * D], in_b[2 * D :]
    SC = 1.0 / np.sqrt(HD)
    w2q = np.ascontiguousarray((wq * SC).T[perm])  # (D rope-feat, D qh-feat)
    w2k = np.ascontiguousarray(wk.T[perm])
    w2T = np.ascontiguousarray(np.concatenate([w2q, w2k], axis=1)).astype(f32)
    b2q = (bq * SC).astype(f32)
    b2k = bk.astype(f32)
    wvT2 = np.ascontiguousarray(wv.T).astype(f32)
    bvr = bv.reshape(1, D).astype(f32)
    owT = np.ascontiguousarray(out_w.T).astype(f32)

    inv_freq = 1.0 / (THETA ** (np.arange(0, D, 2, dtype=np.float64) / D))

    shared = dict(
        w1qkT=w1qkT, w1vT=w1vT, b1qk=b1qk, b1v=b1v.astype(f32),
        w2T=w2T, b2q=b2q, b2k=b2k, wvT=wvT2, bvr=bvr, owT=owT,
        outb=out_b.astype(f32),
    )
    in_maps = []
    for c in range(NCORES):
        rows = slice(TPB * c, TPB * c + TPB)
        xc = np.ascontiguousarray(
            np.concatenate([x[0, rows], x[1, rows]], axis=0).T
        ).astype(f32)
        mc = np.ascontiguousarray(
            np.concatenate([mask[0, rows].T, mask[1, rows].T], axis=1)
        ).astype(f32)
        pos = np.arange(TPB * c, TPB * c + TPB, dtype=np.float64)
        ang = inv_freq[:, None] * pos[None, :]  # (512, 256)
        cosc = np.cos(ang).astype(f32)
        sinc = np.sin(ang).astype(f32)
        m = dict(shared)
        m["xT"] = xc
        m["maskT"] = mc
        m["cosT"] = np.ascontiguousarray(np.concatenate([cosc, cosc], axis=1))
        m["sinT"] = np.ascontiguousarray(np.concatenate([sinc, sinc], axis=1))
        in_maps.append(m)
    return in_maps


def kernel(**inputs):
    if "nc" not in _cached:
        _cached["nc"] = _build_module()
    nc = _cached["nc"]
    in_maps = _host_prep(**inputs)
    res = run_bass_kernel_spmd(nc, in_maps, list(range(NCORES)), trace=TRACE)
    _cached["last_result"] = res
    out = np.empty((B, S, D), dtype=np.float32)
    for c in range(NCORES):
        o = res.results[c]["outT"]  # (D, 512)
        rows = slice(TPB * c, TPB * c + TPB)
        out[0, rows] = o[:, 0:TPB].T
        out[1, rows] = o[:, TPB : 2 * TPB].T
    return out



# revision 8
# speedup vs baseline: 1.2315x; 1.2315x over previous
"""Distributed Trainium2 Bass kernel for nn_Attention (LN + fused QKV + RoPE +
MHA-with-in-proj + out-proj), SPMD over 8 NeuronCores.

Sharding: both batches sequence-sharded across 8 cores; core c owns rows
[256c, 256c+256) of batch 0 AND batch 1 (512 tokens/core). Projections run on
the mixed 512-token block; attention runs per batch (256 queries x 2048 keys).
K-heads and V rows are exchanged via bf16 AllGathers (kh split in two 4MB AGs
so the first head-pairs' scores/exp start earlier; vh one 8MB AG).

Key design points vs the f32r baseline:
 - Everything on the matmul path is bf16 (weights, x, rope, kh/qh/vh, attn),
   f32 PSUM accumulation. Rel err ~1e-2 vs 2e-2 budget (validated in sim).
 - LayerNorm is folded into the qkv projection algebraically:
     qkv = rstd*(W.T x - s1 (x) murstd + b1 (x) sd),  s1 = colsum(W)
   so projections consume RAW x (no LN serialization); the correction is a
   single K=2 matmul per output chunk and the epilogue is one DVE mul.
 - Chain order k -> q -> v; AG(kh pairs 0-3), AG(kh pairs 4-7), AG(vh).
 - Attention: scores [keys, q] per (b, head), exp on ACT (bf16 out), mask
   applied multiplicatively (exp(mask) precomputed once), AV with an appended
   ones-column producing the softmax denominator at partition 64; denominator
   inverted with reciprocal_approx_fast, broadcast on gpsimd.
 - Out-projection per batch so batch-0's out-proj overlaps batch-1 attention.
"""

import numpy as np

import concourse.bass as bass
import concourse.tile as tile
from concourse import bacc, mybir
from concourse.bass_utils import run_bass_kernel_spmd

B, S, D = 2, 2048, 1024
H, HD = 16, 64
NCORES = 8
TPB = 256  # tokens per core per batch
T = 2 * TPB  # tokens per core
EPS = 1e-5
THETA = 10000.0
P = 128
F32 = mybir.dt.float32
BF16 = mybir.dt.bfloat16
Copy = mybir.ActivationFunctionType.Copy
Ident = mybir.ActivationFunctionType.Identity
Exp = mybir.ActivationFunctionType.Exp
Sqrt = mybir.ActivationFunctionType.Sqrt
MUL = mybir.AluOpType.mult
ADD = mybir.AluOpType.add
SUB = mybir.AluOpType.subtract

TRACE = False  # test.py flips this for profiling runs

_cached = {}


def _build_module():
    nc = bacc.Bacc(None, target_bir_lowering=False)

    xT = nc.declare_dram_parameter("xT", [D, T], BF16, isOutput=False)
    maskT = nc.declare_dram_parameter("maskT", [S, T], BF16, isOutput=False)
    cosT = nc.declare_dram_parameter("cosT", [D // 2, T], BF16, isOutput=False)
    sinT = nc.declare_dram_parameter("sinT", [D // 2, T], BF16, isOutput=False)
    w1qkT = nc.declare_dram_parameter("w1qkT", [D, 2 * D], BF16, isOutput=False)
    w1vT = nc.declare_dram_parameter("w1vT", [D, D], BF16, isOutput=False)
    c1qk = nc.declare_dram_parameter("c1qk", [2, 2 * D], BF16, isOutput=False)
    c1v = nc.declare_dram_parameter("c1v", [2, D], BF16, isOutput=False)
    w2T = nc.declare_dram_parameter("w2T", [D, 2 * D], BF16, isOutput=False)
    b2 = nc.declare_dram_parameter("b2", [1, 2 * D], BF16, isOutput=False)
    wvT = nc.declare_dram_parameter("wvT", [D, D], BF16, isOutput=False)
    bvr = nc.declare_dram_parameter("bvr", [1, D], BF16, isOutput=False)
    owT = nc.declare_dram_parameter("owT", [D, D], BF16, isOutput=False)
    outb = nc.declare_dram_parameter("outb", [1, D], BF16, isOutput=False)
    outT = nc.declare_dram_parameter("outT", [D, T], F32, isOutput=True)

    RG = [list(range(NCORES))]

    with tile.TileContext(nc) as tc:
        with (
            tc.tile_pool(name="persist", bufs=1) as persist,
            tc.tile_pool(name="dram", bufs=1, space="DRAM") as dram,
        ):
            qhT = persist.tile([P, 8, T], BF16)  # head-pair-major q heads
            avT = persist.tile([P, 8, T], BF16)  # attention out, feature-major
            expm = persist.tile([P, 16, T], BF16)  # exp(mask), key-major
            cos_sb = persist.tile([P, 4, T], BF16)
            sin_sb = persist.tile([P, 4, T], BF16)
            c1qk_sb = persist.tile([2, 2 * D], BF16)
            c1v_sb = persist.tile([2, D], BF16)
            b2_sb = persist.tile([1, 2 * D], BF16)
            bvr_sb = persist.tile([1, D], BF16)
            outb_sb = persist.tile([1, D], BF16)
            ones_col = persist.tile([P, 1], BF16)
            ones_row = persist.tile([1, T], BF16)
            eps_sb = persist.tile([1, 1], F32)
            corr_rhs = persist.tile([2, T], BF16)  # row0=murstd row1=sd
            rstd_b = persist.tile([P, T], F32)

            nc.vector.memset(ones_col[:], 1.0)
            nc.vector.memset(ones_row[:], 1.0)
            nc.vector.memset(eps_sb[:], EPS)
            nc.sync.dma_start(cos_sb[:], cosT.rearrange("(c p) t -> p c t", p=P))
            nc.sync.dma_start(sin_sb[:], sinT.rearrange("(c p) t -> p c t", p=P))
            nc.sync.dma_start(c1qk_sb[:], c1qk[:])
            nc.sync.dma_start(c1v_sb[:], c1v[:])
            nc.sync.dma_start(b2_sb[:], b2[:])
            nc.sync.dma_start(bvr_sb[:], bvr[:])
            nc.sync.dma_start(outb_sb[:], outb[:])

            ag1a_in = dram.tile([D // 2, T], BF16)
            ag1a_out = dram.tile([NCORES * D // 2, T], BF16, addr_space="Shared")
            ag1b_in = dram.tile([D // 2, T], BF16)
            ag1b_out = dram.tile([NCORES * D // 2, T], BF16, addr_space="Shared")
            ag2_in = dram.tile([T, D], BF16)
            ag2_out = dram.tile([NCORES * T, D], BF16, addr_space="Shared")

            with (
                tc.tile_pool(name="xpool", bufs=1) as xpool,
                tc.tile_pool(name="wpool", bufs=3) as wpool,
                tc.tile_pool(name="psA", bufs=2, space="PSUM") as psA,
            ):
                xfull = xpool.tile([P, 8, T], BF16)
                nc.sync.dma_start(xfull[:], xT.rearrange("(ko p) t -> p ko t", p=P))

                # ---- LN statistics (sum / sum-of-squares via PE) ----
                with (
                    tc.tile_pool(name="sqp", bufs=2) as sqp,
                    tc.tile_pool(name="lnrows", bufs=1) as lnrows,
                    tc.tile_pool(name="psLN", bufs=2, space="PSUM") as psLN,
                ):
                    pt_s = psLN.tile([1, T], F32, tag="s")
                    pt_q = psLN.tile([1, T], F32, tag="q")
                    for ko in range(8):
                        sq = sqp.tile([P, T], BF16, tag="sq")
                        nc.vector.tensor_tensor(
                            sq[:], xfull[:, ko, :], xfull[:, ko, :], MUL
                        )
                        nc.tensor.matmul(
                            pt_s[0:1, :], ones_col[:], xfull[:, ko, :],
                            start=(ko == 0), stop=(ko == 7),
                        )
                        nc.tensor.matmul(
                            pt_q[0:1, :], ones_col[:], sq[:],
                            start=(ko == 0), stop=(ko == 7),
                        )
                    mu = lnrows.tile([1, T], F32)
                    msq = lnrows.tile([1, T], F32)
                    nc.vector.tensor_scalar_mul(mu[:], pt_s[0:1, :], 1.0 / D)
                    nc.vector.tensor_scalar_mul(msq[:], pt_q[0:1, :], 1.0 / D)
                    var = lnrows.tile([1, T], F32)
                    nc.vector.tensor_tensor(var[:], mu[:], mu[:], MUL)
                    nc.vector.tensor_tensor(var[:], msq[:], var[:], SUB)
                    sd = lnrows.tile([1, T], F32)
                    nc.scalar.activation(
                        out=sd[:], in_=var[:], func=Sqrt, bias=eps_sb[:]
                    )
                    rstd = lnrows.tile([1, T], F32)
                    nc.vector.reciprocal_approx_fast(out=rstd[:], in_=sd[:])
                    murstd = lnrows.tile([1, T], F32)
                    nc.vector.tensor_tensor(murstd[:], mu[:], rstd[:], MUL)
                    sdb = lnrows.tile([1, T], BF16)
                    nc.vector.tensor_copy(sdb[:], sd[:])
                    nc.gpsimd.partition_broadcast(corr_rhs[0:2, :], sdb[0:1, :])
                    nc.vector.tensor_copy(corr_rhs[0:1, :], murstd[:])
                    nc.gpsimd.partition_broadcast(rstd_b[:], rstd[:])

                # ---- exp(mask) on ACT (after Sqrt so only one exp set load) --
                with tc.tile_pool(name="mload", bufs=2) as mload:
                    mview = maskT.rearrange("(jc p) t -> p jc t", p=P)
                    for g in range(4):
                        mt = mload.tile([P, 4, T], BF16)
                        nc.sync.dma_start(mt[:], mview[:, 4 * g : 4 * g + 4, :])
                        nc.scalar.activation(
                            out=expm[:, 4 * g : 4 * g + 4, :], in_=mt[:], func=Exp
                        )

                w1view = w1qkT.rearrange("(ko p) j -> p ko j", p=P)
                w1vview = w1vT.rearrange("(ko p) j -> p ko j", p=P)
                w2view = w2T.rearrange("(ko p) j -> p ko j", p=P)
                owview = owT.rearrange("(ko p) j -> p ko j", p=P)

                def project_ln(dst, dst_ko, wview, jcol, corr_sb):
                    """dst[:,dst_ko,:] = rstd*(W.T x + corr.T [murstd; sd])."""
                    wt = wpool.tile([P, 8, P], BF16, tag="w")
                    nc.sync.dma_start(wt[:], wview[:, :, jcol : jcol + P])
                    pt = psA.tile([P, T], F32, tag="proj")
                    for ko in range(8):
                        nc.tensor.matmul(
                            pt[:], wt[:, ko, :], xfull[:, ko, :],
                            start=(ko == 0), stop=False,
                        )
                    nc.tensor.matmul(
                        pt[:], corr_sb[0:2, jcol : jcol + P], corr_rhs[0:2, :],
                        start=False, stop=True,
                    )
                    nc.vector.tensor_tensor(
                        dst[:, dst_ko, :], pt[:], rstd_b[:], MUL
                    )

                def project_plain(dst, dst_ko, wview, jcol, bias_sb, bofs, rhs):
                    """dst[:,dst_ko,:] = W.T rhs + bias."""
                    wt = wpool.tile([P, 8, P], BF16, tag="w")
                    nc.sync.dma_start(wt[:], wview[:, :, jcol : jcol + P])
                    pt = psA.tile([P, T], F32, tag="proj")
                    for ko in range(8):
                        nc.tensor.matmul(
                            pt[:], wt[:, ko, :], rhs[:, ko, :],
                            start=(ko == 0), stop=False,
                        )
                    nc.tensor.matmul(
                        pt[:], bias_sb[0:1, bofs : bofs + P], ones_row[:],
                        start=False, stop=True,
                    )
                    nc.vector.tensor_copy(dst[:, dst_ko, :], pt[:])

                def rope(dst, src, rtmp):
                    for c in range(4):
                        x1 = src[:, c, :]
                        x2 = src[:, 4 + c, :]
                        ta = rtmp.tile([P, T], BF16, tag="ra")
                        tb = rtmp.tile([P, T], BF16, tag="rb")
                        nc.vector.tensor_tensor(ta[:], x1, cos_sb[:, c, :], MUL)
                        nc.vector.tensor_tensor(tb[:], x2, sin_sb[:, c, :], MUL)
                        nc.vector.tensor_tensor(dst[:, c, :], ta[:], tb[:], SUB)
                        tc2 = rtmp.tile([P, T], BF16, tag="ra")
                        td = rtmp.tile([P, T], BF16, tag="rb")
                        nc.vector.tensor_tensor(tc2[:], x2, cos_sb[:, c, :], MUL)
                        nc.vector.tensor_tensor(td[:], x1, sin_sb[:, c, :], MUL)
                        nc.vector.tensor_tensor(
                            dst[:, 4 + c, :], tc2[:], td[:], ADD
                        )

                with (
                    tc.tile_pool(name="qk", bufs=1) as qkp,
                    tc.tile_pool(name="rope", bufs=1) as ropep,
                    tc.tile_pool(name="rtmp", bufs=2) as rtmp,
                    tc.tile_pool(name="stage", bufs=1) as stagep,
                ):
                    # ---- k chain ----
                    kT = qkp.tile([P, 8, T], BF16, tag="qk")
                    for jm in range(8):
                        project_ln(kT, jm, w1view, D + P * jm, c1qk_sb)
                    rk = ropep.tile([P, 8, T], BF16, tag="rope")
                    rope(rk, kT, rtmp)
                    khT = stagep.tile([P, 8, T], BF16, tag="stage")
                    for jm in range(8):
                        project_plain(khT, jm, w2view, D + P * jm, b2_sb, D + P * jm, rk)
                        if jm == 3:
                            nc.sync.dma_start(
                                ag1a_in.rearrange("(ko p) t -> p ko t", p=P),
                                khT[:, 0:4, :],
                            )
                            nc.gpsimd.collective_compute(
                                "AllGather", mybir.AluOpType.bypass,
                                ins=[ag1a_in.opt()], outs=[ag1a_out.opt()],
                                replica_groups=RG,
                            )
                    nc.sync.dma_start(
                        ag1b_in.rearrange("(ko p) t -> p ko t", p=P),
                        khT[:, 4:8, :],
                    )
                    nc.gpsimd.collective_compute(
                        "AllGather", mybir.AluOpType.bypass,
                        ins=[ag1b_in.opt()], outs=[ag1b_out.opt()],
                        replica_groups=RG,
                    )

                    # ---- q chain ----
                    qT = qkp.tile([P, 8, T], BF16, tag="qk")
                    for jm in range(8):
                        project_ln(qT, jm, w1view, P * jm, c1qk_sb)
                    rq = ropep.tile([P, 8, T], BF16, tag="rope")
                    rope(rq, qT, rtmp)
                    for jm in range(8):
                        project_plain(qhT, jm, w2view, P * jm, b2_sb, P * jm, rq)

                    # ---- v chain ----
                    vT = qkp.tile([P, 8, T], BF16, tag="qk")
                    for jm in range(8):
                        project_ln(vT, jm, w1vview, P * jm, c1v_sb)
                    with tc.tile_pool(name="wvp", bufs=2) as wvp:
                        vh_bf = stagep.tile([P, 4, D], BF16, tag="stage")
                        wvview = wvT.rearrange("(ko p) n -> p ko n", p=P)
                        for nh in range(4):
                            wv_rhs = wvp.tile([P, 8, 256], BF16)
                            nc.sync.dma_start(
                                wv_rhs[:], wvview[:, :, 256 * nh : 256 * nh + 256]
                            )
                            for tm in range(4):
                                pt = psA.tile([P, T], F32, tag="proj")
                                for ko in range(8):
                                    nc.tensor.matmul(
                                        pt[:, 0:256],
                                        vT[:, ko, P * tm : P * tm + P],
                                        wv_rhs[:, ko, :],
                                        start=(ko == 0), stop=False,
                                    )
                                nc.tensor.matmul(
                                    pt[:, 0:256],
                                    ones_row[0:1, 0:P],
                                    bvr_sb[0:1, 256 * nh : 256 * nh + 256],
                                    start=False, stop=True,
                                )
                                nc.vector.tensor_copy(
                                    vh_bf[:, tm, 256 * nh : 256 * nh + 256],
                                    pt[:, 0:256],
                                )
                        nc.sync.dma_start(
                            ag2_in.rearrange("(tm p) n -> p tm n", p=P),
                            vh_bf[:],
                        )
                        nc.gpsimd.collective_compute(
                            "AllGather", mybir.AluOpType.bypass,
                            ins=[ag2_in.opt()], outs=[ag2_out.opt()],
                            replica_groups=RG,
                        )

                # ---- attention ----
                # ag1{a,b}_out rows: 512*r + 128*hp' + 64*sub + hd
                # ag2_out rows: 512*r + 256*b + tok ; cols 64*h + hd
                kviewA = ag1a_out.rearrange(
                    "(r hp sub hd) t -> hp (sub hd) r t", hp=4, sub=2, hd=HD
                )
                kviewB = ag1b_out.rearrange(
                    "(r hp sub hd) t -> hp (sub hd) r t", hp=4, sub=2, hd=HD
                )
                vview = ag2_out.rearrange(
                    "(r b2 half p) f -> b2 half p r f", b2=2, half=2, p=P
                )
                with (
                    tc.tile_pool(name="kload", bufs=2) as kload,
                    tc.tile_pool(name="vload", bufs=2) as vload,
                    tc.tile_pool(name="apool", bufs=2) as apool,
                    tc.tile_pool(name="nrm", bufs=2) as nrm,
                    tc.tile_pool(name="oc", bufs=2) as ocp,
                    tc.tile_pool(name="psS", bufs=2, space="PSUM") as psS,
                    tc.tile_pool(name="psV", bufs=2, space="PSUM") as psV,
                ):
                    for b in range(2):
                        for hp in range(8):
                            kview = kviewA if hp < 4 else kviewB
                            kp = kload.tile([P, 8, TPB], BF16)
                            nc.sync.dma_start(
                                kp[:], kview[hp % 4][:, :, TPB * b : TPB * b + TPB]
                            )
                            vh_sb = vload.tile([P, 8, 2, 2, HD + 1], BF16)
                            for half in range(2):
                                for sub in range(2):
                                    c0 = P * hp + HD * sub
                                    nc.sync.dma_start(
                                        vh_sb[:, :, half, sub, 0:HD],
                                        vview[b][half][:, :, c0 : c0 + HD],
                                    )
                            nc.vector.memset(vh_sb[:, :, :, :, HD : HD + 1], 1.0)

                            for sub in range(2):
                                h0 = HD * sub
                                av_pt = psV.tile([P, TPB], F32, tag="av")
                                for g in range(4):
                                    s_pt = psS.tile([P, 4, TPB], F32, tag="s")
                                    for u in range(4):
                                        jc = 4 * g + u
                                        r, half = jc // 2, jc % 2
                                        nc.tensor.matmul(
                                            s_pt[:, u, :],
                                            kp[h0 : h0 + HD, r, P * half : P * half + P],
                                            qhT[h0 : h0 + HD, hp, TPB * b : TPB * b + TPB],
                                            start=True, stop=True,
                                        )
                                    attnE = apool.tile([P, 4, TPB], BF16, tag="ae")
                                    nc.scalar.activation(
                                        out=attnE[:], in_=s_pt[:], func=Exp
                                    )
                                    attnT = apool.tile([P, 4, TPB], BF16, tag="at")
                                    nc.vector.tensor_tensor(
                                        attnT[:],
                                        attnE[:],
                                        expm[:, 4 * g : 4 * g + 4, TPB * b : TPB * b + TPB],
                                        MUL,
                                    )
                                    for u in range(4):
                                        jc = 4 * g + u
                                        nc.tensor.matmul(
                                            av_pt[0 : HD + 1, :],
                                            vh_sb[:, jc // 2, jc % 2, sub, 0 : HD + 1],
                                            attnT[:, u, :],
                                            start=(g == 0 and u == 0),
                                            stop=(g == 3 and u == 3),
                                        )
                                # normalize by the ones-column denominator
                                avs = nrm.tile([P, TPB], F32, tag="avs")
                                nc.vector.tensor_copy(
                                    avs[0 : HD + 1, :], av_pt[0 : HD + 1, :]
                                )
                                drow = nrm.tile([1, TPB], F32, tag="dr")
                                nc.sync.dma_start(drow[:], avs[HD : HD + 1, :])
                                rrow = nrm.tile([1, TPB], F32, tag="rr")
                                nc.vector.reciprocal_approx_fast(
                                    out=rrow[:], in_=drow[:]
                                )
                                rb = nrm.tile([HD, TPB], F32, tag="rbt")
                                nc.gpsimd.partition_broadcast(rb[:], rrow[:])
                                if sub == 0:
                                    nc.vector.tensor_tensor(
                                        avT[0:HD, hp, TPB * b : TPB * b + TPB],
                                        avs[0:HD, :], rb[:], MUL,
                                    )
                                else:
                                    avn = nrm.tile([HD, TPB], BF16, tag="avn")
                                    nc.vector.tensor_tensor(
                                        avn[:], avs[0:HD, :], rb[:], MUL
                                    )
                                    nc.sync.dma_start(
                                        avT[HD:P, hp, TPB * b : TPB * b + TPB],
                                        avn[:],
                                    )

                        # ---- out-projection for this batch ----
                        for om in range(8):
                            owt = wpool.tile([P, 8, P], BF16, tag="w")
                            nc.sync.dma_start(
                                owt[:], owview[:, :, P * om : P * om + P]
                            )
                            pt = psA.tile([P, T], F32, tag="proj")
                            for ko in range(8):
                                nc.tensor.matmul(
                                    pt[:, 0:TPB],
                                    owt[:, ko, :],
                                    avT[:, ko, TPB * b : TPB * b + TPB],
                                    start=(ko == 0), stop=False,
                                )
                            nc.tensor.matmul(
                                pt[:, 0:TPB],
                                outb_sb[0:1, P * om : P * om + P],
                                ones_row[0:1, 0:TPB],
                                start=False, stop=True,
                            )
                            oc = ocp.tile([P, TPB], F32, tag="oc")
                            nc.vector.tensor_copy(oc[:], pt[:, 0:TPB])
                            nc.sync.dma_start(
                                outT.rearrange("(ko p) t -> p ko t", p=P)[
                                    :, om, TPB * b : TPB * b + TPB
                                ],
                                oc[:],
                            )

    nc.finalize()
    return nc


def _bf16(x):
    x = np.ascontiguousarray(np.asarray(x, np.float32))
    u = x.view(np.uint32)
    r = ((u >> 16) & 1).astype(np.uint32)
    return ((u + 0x7FFF + r) & 0xFFFF0000).view(np.float32)


def _host_prep(x, mask, ln_g, ln_b, w_qkv, b_qkv, in_w, in_b, out_w, out_b):
    f32 = np.float32
    import ml_dtypes

    def to_bf(a):
        return np.asarray(a, np.float32).astype(ml_dtypes.bfloat16)

    perm = np.concatenate([np.arange(0, D, 2), np.arange(1, D, 2)])
    W1 = (w_qkv * ln_g[None, :]).astype(f32)
    b1 = (b_qkv + w_qkv @ ln_b).astype(f32)
    W1q, W1k, W1v = W1[0:D], W1[D : 2 * D], W1[2 * D :]
    b1q, b1k, b1v = b1[0:D], b1[D : 2 * D], b1[2 * D :]
    w1qkT = _bf16(np.concatenate([W1q[perm], W1k[perm]], axis=0).T)  # (D,2D)
    c1qk = np.stack(
        [-w1qkT.sum(axis=0), np.concatenate([b1q[perm], b1k[perm]])]
    ).astype(f32)  # (2, 2D): row0=-s1 row1=b1
    w1vT = _bf16(W1v.T)
    c1v = np.stack([-w1vT.sum(axis=0), b1v]).astype(f32)

    wq, wk, wv = in_w[0:D], in_w[D : 2 * D], in_w[2 * D :]
    bq, bk, bv = in_b[0:D], in_b[D : 2 * D], in_b[2 * D :]
    SC = 1.0 / np.sqrt(HD)
    w2q = np.ascontiguousarray((wq * SC).T[perm])
    w2k = np.ascontiguousarray(wk.T[perm])
    w2T = np.concatenate([w2q, w2k], axis=1).astype(f32)
    b2r = np.concatenate([bq * SC, bk]).reshape(1, 2 * D).astype(f32)
    wvT2 = np.ascontiguousarray(wv.T).astype(f32)
    bvr = bv.reshape(1, D).astype(f32)
    owT = np.ascontiguousarray(out_w.T).astype(f32)
    outbr = out_b.reshape(1, D).astype(f32)

    inv_freq = 1.0 / (THETA ** (np.arange(0, D, 2, dtype=np.float64) / D))

    shared = dict(
        w1qkT=to_bf(w1qkT), w1vT=to_bf(w1vT), c1qk=to_bf(c1qk), c1v=to_bf(c1v),
        w2T=to_bf(w2T), b2=to_bf(b2r), wvT=to_bf(wvT2), bvr=to_bf(bvr),
        owT=to_bf(owT), outb=to_bf(outbr),
    )
    in_maps = []
    for c in range(NCORES):
        rows = slice(TPB * c, TPB * c + TPB)
        xc = np.ascontiguousarray(
            np.concatenate([x[0, rows], x[1, rows]], axis=0).T
        )
        mc = np.ascontiguousarray(
            np.concatenate([mask[0, rows].T, mask[1, rows].T], axis=1)
        )
        pos = np.arange(TPB * c, TPB * c + TPB, dtype=np.float64)
        ang = inv_freq[:, None] * pos[None, :]  # (512, 256)
        cosc = np.cos(ang)
        sinc = np.sin(ang)
        m = dict(shared)
        m["xT"] = to_bf(xc)
        m["maskT"] = to_bf(mc)
        m["cosT"] = to_bf(np.concatenate([cosc, cosc], axis=1))
        m["sinT"] = to_bf(np.concatenate([sinc, sinc], axis=1))
        in_maps.append(m)
    return in_maps


def kernel(**inputs):
    if "nc" not in _cached:
        _cached["nc"] = _build_module()
    nc = _cached["nc"]
    in_maps = _host_prep(**inputs)
    res = run_bass_kernel_spmd(nc, in_maps, list(range(NCORES)), trace=TRACE)
    _cached["last_result"] = res
    out = np.empty((B, S, D), dtype=np.float32)
    for c in range(NCORES):
        o = res.results[c]["outT"]  # (D, 512)
        rows = slice(TPB * c, TPB * c + TPB)
        out[0, rows] = o[:, 0:TPB].T
        out[1, rows] = o[:, TPB : 2 * TPB].T
    return out
